# revision 1
# baseline (speedup 1.0000x reference)
"""Trainium2 Bass kernel for nn_AttentionBlock (GroupNorm + 4-head attention + proj + residual).

Problem (hardcoded): x [4, 256, 64, 64] f32, 32 groups, 4 heads (ch=64/head),
T = 64*64 = 4096 tokens per batch item.

Sharding over 8 NeuronCores: core = (batch b, query-half). Each core receives
x[b] with its token-half rotated to the front (attention is invariant to a
consistent permutation of key/value order, and queries are sliced), computes
GroupNorm(x[b]) + full K/V for all 4 heads, flash-attention for its 2048
queries, proj + residual for those tokens. Outputs are disjoint -> the host
just stitches halves back together. No collectives needed.

Inside a core everything is bf16 on the PE (f32 PSUM accumulation); softmax
runs unnormalized (exp on ScalarE, row-sums via an appended ones-column in
the P@V matmul, normalization folded in afterwards on VectorE).
"""

import functools
import os

import numpy as np

import concourse.bass as bass
import concourse.tile as tile
from concourse import bacc, mybir
from concourse.bass_utils import run_bass_kernel_spmd

F32 = mybir.dt.float32
BF16 = mybir.dt.bfloat16
AF = mybir.ActivationFunctionType
OP = mybir.AluOpType

P = 128          # partitions
C = 256          # channels
T = 4096         # tokens per batch item
TQ = 2048        # query tokens per core (half of T)
CH = 64          # channels per head
GS = 8           # channels per group
EPS = 1e-5
SCALE = 0.125    # 1/sqrt(ch) applied inside exp
N_CORES = 8


def _emit(nc, tc, aps):
    xp, wqkv, bqkv, wproj, bproj, gamma, beta, outp = aps

    with (
        tc.tile_pool(name="pp", bufs=1) as pp,
        tc.tile_pool(name="wk", bufs=2) as wk,
        tc.tile_pool(name="ptp", bufs=6) as ptp,
        tc.tile_pool(name="ps", bufs=2, space="PSUM") as ps,
    ):
        # ---------------- constants (NEFF-embedded, no gpsimd) ----------------
        import ml_dtypes
        ident_np = np.eye(P, dtype=ml_dtypes.bfloat16)
        selg_np = (np.arange(P)[:, None] // GS == np.arange(16)[None, :]).astype(np.float32)
        ident_dram = nc.inline_tensor(ident_np, name="ident_c")
        selg_dram = nc.inline_tensor(selg_np, name="selg_c")
        selgT_dram = nc.inline_tensor(np.ascontiguousarray(selg_np.T), name="selgT_c")

        ident = pp.tile([P, P], BF16)
        nc.sync.dma_start(out=ident, in_=ident_dram.ap())
        # selg[p, g] = 1 iff p//8 == g  (sums 8 consecutive partitions)
        selg = pp.tile([P, 16], F32)
        nc.sync.dma_start(out=selg, in_=selg_dram.ap())
        # selgT[g, p] = 1 iff p//8 == g  (broadcasts group value to 8 partitions)
        selgT = pp.tile([16, P], F32)
        nc.sync.dma_start(out=selgT, in_=selgT_dram.ap())

        eps16 = pp.tile([16, 1], F32)
        nc.vector.memset(eps16, EPS)
        ones64 = pp.tile([1, 64], F32)
        nc.vector.memset(ones64, 1.0)

        # ---------------- input DMAs ----------------
        x_sb = [pp.tile([P, T], F32, name=f"x_sb{ct}") for ct in range(2)]
        for ct in range(2):
            for chk in range(4):
                nc.sync.dma_start(
                    out=x_sb[ct][:, 1024 * chk:1024 * (chk + 1)],
                    in_=xp[P * ct:P * (ct + 1), 1024 * chk:1024 * (chk + 1)])

        wq_st = pp.tile([P, 6, C], F32)
        nc.sync.dma_start(out=wq_st, in_=wqkv.rearrange("(a p) c -> p a c", p=P))
        wp_st = pp.tile([P, 2, C], F32)
        nc.sync.dma_start(out=wp_st, in_=wproj.rearrange("(a p) c -> p a c", p=P))

        bq = [pp.tile([P, 1], F32, name=f"bq{p_}") for p_ in range(2)]
        bk = [pp.tile([P, 1], F32, name=f"bk{p_}") for p_ in range(2)]
        bv = [pp.tile([P, 1], F32, name=f"bv{p_}") for p_ in range(2)]
        for pr in range(2):
            for a in range(2):
                h = 2 * pr + a
                sl = slice(64 * a, 64 * (a + 1))
                nc.sync.dma_start(out=bq[pr][sl, :], in_=bqkv[192 * h:192 * h + 64, :])
                nc.sync.dma_start(out=bk[pr][sl, :], in_=bqkv[192 * h + 64:192 * h + 128, :])
                nc.sync.dma_start(out=bv[pr][sl, :], in_=bqkv[192 * h + 128:192 * h + 192, :])
        gam = [pp.tile([P, 1], F32, name=f"gam{ct}") for ct in range(2)]
        bet = [pp.tile([P, 1], F32, name=f"bet{ct}") for ct in range(2)]
        bp = [pp.tile([P, 1], F32, name=f"bp{ct}") for ct in range(2)]
        for ct in range(2):
            sl = slice(P * ct, P * (ct + 1))
            nc.sync.dma_start(out=gam[ct], in_=gamma[sl, :])
            nc.sync.dma_start(out=bet[ct], in_=beta[sl, :])
            nc.sync.dma_start(out=bp[ct], in_=bproj[sl, :])

        # ---------------- GroupNorm statistics ----------------
        # Per-channel mean/var via bn_stats, then 8-channel group sums via a
        # tiny f32 selector matmul, then broadcast back the same way.
        rhs_stats = pp.tile([P, 4], F32)
        for ct in range(2):
            xv = x_sb[ct].rearrange("p (n f) -> p n f", f=512)
            stats = wk.tile([P, 8, 6], F32, tag="bnstats")
            for i in range(8):
                nc.vector.bn_stats(out=stats[:, i, :], in_=xv[:, i, :])
            mv = wk.tile([P, 2], F32, tag="bnmv")
            nc.vector.bn_aggr(out=mv, in_=stats)
            nc.vector.tensor_copy(out=rhs_stats[:, 2 * ct:2 * ct + 1], in_=mv[:, 0:1])
            m2 = wk.tile([P, 1], F32, tag="m2")
            nc.vector.tensor_mul(out=m2, in0=mv[:, 0:1], in1=mv[:, 0:1])
            nc.vector.tensor_add(out=rhs_stats[:, 2 * ct + 1:2 * ct + 2],
                                 in0=mv[:, 1:2], in1=m2)

        gst_ps = ps.tile([16, 4], F32, tag="st")
        nc.tensor.matmul(out=gst_ps, lhsT=selg, rhs=rhs_stats, start=True, stop=True)
        gstat = pp.tile([16, 4], F32)
        nc.vector.tensor_scalar_mul(out=gstat, in0=gst_ps, scalar1=1.0 / GS)
        g3 = gstat.rearrange("p (a b) -> p a b", b=2)
        mu2 = pp.tile([16, 2], F32)
        nc.vector.tensor_mul(out=mu2, in0=g3[:, :, 0], in1=g3[:, :, 0])
        var_t = pp.tile([16, 2], F32)
        nc.vector.tensor_sub(out=var_t, in0=g3[:, :, 1], in1=mu2)
        # rstd via Sqrt + DVE reciprocal: both ACT table loads (sqrt set here,
        # exp set before the first attention exp) hide in ACT idle time.
        std_t = pp.tile([16, 2], F32)
        nc.scalar.activation(out=std_t, in_=var_t, func=AF.Sqrt, bias=eps16, scale=1.0)
        rs_t = pp.tile([16, 2], F32)
        nc.vector.reciprocal(out=rs_t, in_=std_t)

        A_t = [pp.tile([P, 1], F32, name=f"A_t{ct}") for ct in range(2)]
        B_t = [pp.tile([P, 1], F32, name=f"B_t{ct}") for ct in range(2)]
        for ct in range(2):
            rhs_bc = wk.tile([16, 2], F32, tag="rhsbc")
            nc.vector.tensor_copy(out=rhs_bc[:, 0:1], in_=gstat[:, 2 * ct:2 * ct + 1])
            nc.vector.tensor_copy(out=rhs_bc[:, 1:2], in_=rs_t[:, ct:ct + 1])
            bc_ps = ps.tile([P, 2], F32, tag="st", name=f"bc_ps{ct}")
            nc.tensor.matmul(out=bc_ps, lhsT=selgT, rhs=rhs_bc, start=True, stop=True)
            nc.vector.tensor_mul(out=A_t[ct], in0=bc_ps[:, 1:2], in1=gam[ct])
            tb = wk.tile([P, 1], F32, tag="tb")
            nc.vector.tensor_mul(out=tb, in0=bc_ps[:, 0:1], in1=A_t[ct])
            nc.vector.tensor_sub(out=B_t[ct], in0=bet[ct], in1=tb)

        # h = x * A + B   (A = rstd*gamma, B = beta - mu*rstd*gamma), cast bf16
        h_bf = [pp.tile([P, T], BF16, name=f"h_bf{ct}") for ct in range(2)]
        for chk in range(4):
            csl = slice(1024 * chk, 1024 * (chk + 1))
            for ct in range(2):
                nc.vector.tensor_scalar(out=h_bf[ct][:, csl], in0=x_sb[ct][:, csl],
                                        scalar1=A_t[ct], scalar2=B_t[ct],
                                        op0=OP.mult, op1=OP.add)

        # residual + proj bias staged into the output buffer
        out_sb = [pp.tile([P, TQ], F32, name=f"out_sb{ct}") for ct in range(2)]
        for ct in range(2):
            nc.vector.tensor_scalar_add(out=out_sb[ct], in0=x_sb[ct][:, 0:TQ],
                                        scalar1=bp[ct])

        # ---------------- weight transposes (PE identity matmuls) ----------------
        wq_bf = pp.tile([P, 6, C], BF16)
        nc.vector.tensor_copy(out=wq_bf, in_=wq_st)
        wp_bf = pp.tile([P, 2, C], BF16)
        nc.vector.tensor_copy(out=wp_bf, in_=wp_st)
        # WTq column layout: [q_p0 | q_p1 | k_p0 | k_p1 | v_p0 | v_p1], each a
        # contiguous 128-col block (pair = heads 2p,2p+1), so matmul operand
        # slices are single-free-dim APs.
        WTq = [pp.tile([P, 768], BF16, name=f"WTq{j}") for j in range(2)]
        WTp = [pp.tile([P, C], BF16, name=f"WTp{j}") for j in range(2)]
        for i in range(6):
            for j in range(2):
                tq_ps = ps.tile([P, P], BF16, tag="st", name=f"tq_ps{i}{j}")
                nc.tensor.transpose(out=tq_ps, in_=wq_bf[:, i, P * j:P * (j + 1)],
                                    identity=ident)
                for a in range(2):
                    o0 = P * i + 64 * a
                    h = o0 // 192
                    kind = (o0 % 192) // 64
                    dcol = kind * 256 + (h // 2) * 128 + (h % 2) * 64
                    nc.vector.tensor_copy(out=WTq[j][:, dcol:dcol + 64],
                                          in_=tq_ps[:, 64 * a:64 * (a + 1)])
        for i in range(2):
            for j in range(2):
                tp_ps = ps.tile([P, P], BF16, tag="st", name=f"tp_ps{i}{j}")
                nc.tensor.transpose(out=tp_ps, in_=wp_bf[:, i, P * j:P * (j + 1)],
                                    identity=ident)
                nc.vector.tensor_copy(out=WTp[j][:, P * i:P * (i + 1)], in_=tp_ps)

        def wt_slice(j, kind, pr):
            base = kind * 256 + pr * 128
            return WTq[j][:, base:base + 128]

        # ---------------- QKV projections ----------------
        q_sb = [pp.tile([P, TQ], BF16, name=f"q_sb{p_}") for p_ in range(2)]
        k_sb = [pp.tile([P, T], BF16, name=f"k_sb{p_}") for p_ in range(2)]
        vT_sb = [pp.tile([P, 32, 2, 65], BF16, name=f"vT_sb{p_}") for p_ in range(2)]

        def gen_qkv_stripe(pr, tt):
            # Two-part stripe emission (k+q, then v) so slot demand on the
            # shared "st" psum tag spreads across attention pipeline steps.
            it_ = iter(_qkv_stripe_parts(pr, tt))
            return it_

        def emit_qkv_stripe(pr, tt):
            for _ in _qkv_stripe_parts(pr, tt):
                pass

        def _qkv_stripe_parts(pr, tt):
            # One 512-column stripe: k (+q for tt<4) and vT for pair pr.
            # k/q share one st-tagged psum tile (2 banks) to limit slot churn
            # while interleaved with the attention pipeline.
            if tt == 0:
                nc.vector.memset(vT_sb[pr][:, :, :, 64:65], 1.0)
            tsl = slice(512 * tt, 512 * (tt + 1))
            s1 = ps.tile([P, 2, 512], F32, tag="st", name=f"qk_ps{pr}{tt}")
            for ct in range(2):
                nc.tensor.matmul(out=s1[:, 0, :], lhsT=wt_slice(ct, 1, pr),
                                 rhs=h_bf[ct][:, tsl],
                                 start=(ct == 0), stop=(ct == 1))
            nc.vector.tensor_scalar_add(out=k_sb[pr][:, tsl], in0=s1[:, 0, :],
                                        scalar1=bk[pr])
            if tt < 4:
                for ct in range(2):
                    nc.tensor.matmul(out=s1[:, 1, :], lhsT=wt_slice(ct, 0, pr),
                                     rhs=h_bf[ct][:, tsl],
                                     start=(ct == 0), stop=(ct == 1))
                nc.vector.tensor_scalar_add(out=q_sb[pr][:, tsl],
                                            in0=s1[:, 1, :], scalar1=bq[pr])
                yield
                vtile = ps.tile([P, 2, 512], F32, tag="st", name=f"v_ps{pr}{tt}")
                vsl = vtile[:, 0, :]
            else:
                yield
                vsl = s1[:, 1, :]
            vv = vsl.rearrange("p (j n) -> p j n", j=4)
            for j in range(4):
                it = 4 * tt + j
                for ct in range(2):
                    nc.tensor.matmul(out=vv[:, j, :],
                                     lhsT=h_bf[ct][:, P * it:P * (it + 1)],
                                     rhs=wt_slice(ct, 2, pr),
                                     start=(ct == 0), stop=(ct == 1))
            nc.vector.tensor_copy(
                out=vT_sb[pr][:, 4 * tt:4 * (tt + 1), :, 0:64],
                in_=vsl.rearrange("p (j h c) -> p j h c", j=4, h=2))
            yield

        # ---------------- attention ----------------
        # KSTAGE: debug knob — 2 = stop after qkv, 3 = one attention tile only.
        kstage = int(os.environ.get("KSTAGE", "9"))
        a_sb = [pp.tile([P, TQ], BF16, name=f"a_sb{p_}") for p_ in range(2)]
        if kstage <= 3:
            for pr_ in range(2):
                nc.vector.memset(a_sb[pr_], 0.0)
        n_tt = 0 if kstage <= 2 else (1 if kstage == 3 else 4)

        gens_pv = {}

        def gen_attn(pr, tt):
            """Pipeline-step generator for one (pair, q-tile): step s emits the
            two ST matmuls + exp for key-block s (s<32) and the PV matmuls for
            block s-1 — ST_{s+1} always precedes PV_s in the in-order PE stream
            so ACT (the bottleneck) never starves."""
            tsl = slice(512 * tt, 512 * (tt + 1))
            pv = [ps.tile([65, 512], F32, tag="pv", bufs=3,
                          name=f"pv{pr}{tt}{h}") for h in range(2)]
            gens_pv[(pr, tt)] = pv
            pts = {}
            for s in range(33):
                if s < 32:
                    st = ps.tile([P, 2, 512], F32, tag="st", bufs=2,
                                 name=f"st{pr}{tt}{s}")
                    for h in range(2):
                        nc.tensor.matmul(
                            out=st[:, h, :],
                            lhsT=k_sb[pr][64 * h:64 * (h + 1), P * s:P * (s + 1)],
                            rhs=q_sb[pr][64 * h:64 * (h + 1), tsl],
                            start=True, stop=True)
                    pt = ptp.tile([P, 2, 512], BF16, tag="pt",
                                  name=f"pt{pr}{tt}{s}")
                    nc.scalar.activation(out=pt, in_=st, func=AF.Exp, scale=SCALE)
                    pts[s] = pt
                if s >= 1:
                    pt_prev = pts.pop(s - 1)
                    for h in range(2):
                        nc.tensor.matmul(out=pv[h],
                                         lhsT=vT_sb[pr][:, s - 1, h, :],
                                         rhs=pt_prev[:, h, :],
                                         start=(s == 1), stop=(s == 32))
                yield

        def emit_normalize(pr, tt):
            # a = pv[0:64] / Z + v_bias ; Z sits in row 64. 1/Z broadcast to
            # 64 partitions via a K=1 f32 ones-matmul.
            pv = gens_pv.pop((pr, tt))
            tsl = slice(512 * tt, 512 * (tt + 1))
            for h in range(2):
                rz = wk.tile([1, 512], F32, tag="rz")
                nc.vector.reciprocal(out=rz, in_=pv[h][64:65, :])
                rzb_ps = ps.tile([64, 512], F32, tag="bc", bufs=1,
                                 name=f"rzb_ps{pr}{tt}{h}")
                nc.tensor.matmul(out=rzb_ps, lhsT=ones64, rhs=rz,
                                 start=True, stop=True)
                rzb = wk.tile([64, 512], F32, tag="rzb")
                nc.vector.tensor_copy(out=rzb, in_=rzb_ps)
                asl = a_sb[pr][64 * h:64 * (h + 1), tsl]
                nc.vector.tensor_mul(out=asl, in0=pv[h][0:64, :], in1=rzb)
                nc.vector.tensor_scalar_add(out=asl, in0=asl,
                                            scalar1=bv[pr][64 * h:64 * (h + 1), :])

        proj_done = set()

        def emit_proj_tt(tt):
            # proj + residual for one 512-column stripe (both output chunks);
            # psums borrow the "bc" tag, idle outside normalize.
            proj_done.add(tt)
            tsl = slice(512 * tt, 512 * (tt + 1))
            for oc in range(2):
                pj = ps.tile([P, 512], F32, tag="bc", bufs=1, name=f"pj{oc}{tt}")
                for ct in range(2):
                    nc.tensor.matmul(out=pj, lhsT=WTp[ct][:, P * oc:P * (oc + 1)],
                                     rhs=a_sb[ct][:, tsl],
                                     start=(ct == 0), stop=(ct == 1))
                nc.vector.tensor_add(out=out_sb[oc][:, tsl],
                                     in0=out_sb[oc][:, tsl], in1=pj)
                nc.sync.dma_start(out=outp[P * oc:P * (oc + 1), tsl],
                                  in_=out_sb[oc][:, tsl])

        def drive(g, n):
            for _ in range(n):
                next(g, None)

        # ---- flat schedule: qkv stripes interleaved into attention steps ----
        emit_qkv_stripe(0, 0)
        tiles = [(0, t_) for t_ in range(n_tt)]
        if kstage >= 4:
            tiles += [(1, t_) for t_ in range(n_tt)]
        prev = None
        for idx, (pr, tt) in enumerate(tiles):
            g = gen_attn(pr, tt)
            if idx == 0:
                # remaining p0 stripes, split k+q / v across each 4-step
                # window; ST block s only needs stripes up to s//4, already
                # fully emitted by the window boundary.
                for i in range(1, 8):
                    sp = gen_qkv_stripe(0, i)
                    drive(g, 2)
                    next(sp, None)
                    drive(g, 2)
                    next(sp, None)
                    next(sp, None)
                drive(g, 5)
            else:
                drive(g, 2)          # ST0/exp0/ST1/exp1/PV0 queued for ACT
                if prev is not None:
                    emit_normalize(*prev)
                drive(g, 1)
                if prev is not None and prev[0] == 1:
                    emit_proj_tt(prev[1])
                if kstage >= 4 and (pr, tt) == (0, 3):
                    done = 3         # p1 stripes hidden under tile (0,3)
                    for si in range(8):
                        sp = gen_qkv_stripe(1, si)
                        mid = min(4 * si + 2, 33)
                        drive(g, max(0, mid - done))
                        done = mid
                        next(sp, None)
                        target = min(4 * (si + 1), 33)
                        drive(g, max(0, target - done))
                        done = target
                        next(sp, None)
                        next(sp, None)
                    drive(g, 33 - done)
                else:
                    drive(g, 30)
            prev = (pr, tt)
        if prev is not None:
            emit_normalize(*prev)
        if kstage >= 4 and kstage < 9:
            pass
        if kstage < 4:
            emit_qkv_stripe(1, 0)    # keep p1 buffers initialized
        for t_ in range(4):
            if t_ not in proj_done:
                emit_proj_tt(t_)


@functools.cache
def _build():
    nc = bacc.Bacc("TRN2", target_bir_lowering=False, debug=False)
    xp = nc.dram_tensor("xp", [C, T], F32, kind="ExternalInput").ap()
    wqkv = nc.dram_tensor("wqkv", [3 * C, C], F32, kind="ExternalInput").ap()
    bqkv = nc.dram_tensor("bqkv", [3 * C, 1], F32, kind="ExternalInput").ap()
    wproj = nc.dram_tensor("wproj", [C, C], F32, kind="ExternalInput").ap()
    bproj = nc.dram_tensor("bproj", [C, 1], F32, kind="ExternalInput").ap()
    gamma = nc.dram_tensor("gamma", [C, 1], F32, kind="ExternalInput").ap()
    beta = nc.dram_tensor("beta", [C, 1], F32, kind="ExternalInput").ap()
    outp = nc.dram_tensor("outp", [C, TQ], F32, kind="ExternalOutput").ap()
    with tile.TileContext(nc) as tc:
        _emit(nc, tc, (xp, wqkv, bqkv, wproj, bproj, gamma, beta, outp))
    nc.finalize()
    return nc


def _make_in_maps(x, gamma, beta, w_qkv, qkv_bias, w_proj, proj_bias):
    xf = np.ascontiguousarray(np.asarray(x, np.float32)).reshape(4, C, T)
    shared = {
        "wqkv": np.ascontiguousarray(np.asarray(w_qkv, np.float32)),
        "bqkv": np.ascontiguousarray(np.asarray(qkv_bias, np.float32).reshape(3 * C, 1)),
        "wproj": np.ascontiguousarray(np.asarray(w_proj, np.float32)),
        "bproj": np.ascontiguousarray(np.asarray(proj_bias, np.float32).reshape(C, 1)),
        "gamma": np.ascontiguousarray(np.asarray(gamma, np.float32).reshape(C, 1)),
        "beta": np.ascontiguousarray(np.asarray(beta, np.float32).reshape(C, 1)),
    }
    in_maps = []
    for core in range(N_CORES):
        b, half = divmod(core, 2)
        if half == 0:
            xpc = xf[b]
        else:
            xpc = np.concatenate([xf[b][:, TQ:], xf[b][:, :TQ]], axis=1)
        in_maps.append({"xp": np.ascontiguousarray(xpc), **shared})
    return in_maps


def _run(in_maps, **kwargs):
    nc = _build()
    return run_bass_kernel_spmd(nc, in_maps, core_ids=list(range(N_CORES)), **kwargs)


def kernel(x, gamma, beta, w_qkv, qkv_bias, w_proj, proj_bias, num_heads):
    assert int(num_heads) == 4
    in_maps = _make_in_maps(x, gamma, beta, w_qkv, qkv_bias, w_proj, proj_bias)
    res = _run(in_maps)
    out = np.empty((4, C, T), np.float32)
    for core in range(N_CORES):
        b, half = divmod(core, 2)
        out[b][:, half * TQ:(half + 1) * TQ] = res.results[core]["outp"]
    return out.reshape(4, C, 64, 64)



# revision 4
# speedup vs baseline: 1.0251x; 1.0251x over previous
"""Trainium2 Bass kernel for nn_AttentionBlock (GroupNorm + 4-head attention + proj + residual).

Problem (hardcoded): x [4, 256, 64, 64] f32, 32 groups, 4 heads (ch=64/head),
T = 64*64 = 4096 tokens per batch item.

Sharding over 8 NeuronCores: core = (batch b, query-half). Each core receives
x[b] with its token-half rotated to the front (attention is invariant to a
consistent permutation of key/value order, and queries are sliced), computes
GroupNorm(x[b]) + full K/V for all 4 heads, flash-attention for its 2048
queries, proj + residual for those tokens. Outputs are disjoint -> the host
just stitches halves back together. No collectives needed.

Inside a core everything is bf16 on the PE (f32 PSUM accumulation); softmax
runs unnormalized (exp on ScalarE, row-sums via an appended ones-column in
the P@V matmul, normalization folded in afterwards on VectorE).
"""

import functools
import os

import numpy as np

import concourse.bass as bass
import concourse.tile as tile
from concourse import bacc, mybir
from concourse.bass_utils import run_bass_kernel_spmd

F32 = mybir.dt.float32
BF16 = mybir.dt.bfloat16
AF = mybir.ActivationFunctionType
OP = mybir.AluOpType

P = 128          # partitions
C = 256          # channels
T = 4096         # tokens per batch item
TQ = 2048        # query tokens per core (half of T)
CH = 64          # channels per head
GS = 8           # channels per group
EPS = 1e-5
SCALE = 0.125    # 1/sqrt(ch) applied inside exp
N_CORES = 8


def _emit(nc, tc, aps):
    xp, wqkv, bqkv, wproj, bproj, gamma, beta, outp = aps

    with (
        tc.tile_pool(name="pp", bufs=1) as pp,
        tc.tile_pool(name="wk", bufs=2) as wk,
        tc.tile_pool(name="ptp", bufs=6) as ptp,
        tc.tile_pool(name="ps", bufs=2, space="PSUM") as ps,
    ):
        # ---------------- constants (NEFF-embedded, no gpsimd) ----------------
        import ml_dtypes
        ident_np = np.eye(P, dtype=ml_dtypes.bfloat16)
        selg_np = (np.arange(P)[:, None] // GS == np.arange(16)[None, :]).astype(np.float32)
        ident_dram = nc.inline_tensor(ident_np, name="ident_c")
        selg_dram = nc.inline_tensor(selg_np, name="selg_c")
        selgT_dram = nc.inline_tensor(np.ascontiguousarray(selg_np.T), name="selgT_c")

        # Weights/consts ride the ACT queue so they land ~t=0 (the big x
        # transfers own the SP queue); PE transposes then start immediately.
        ident = pp.tile([P, P], BF16)
        nc.scalar.dma_start(out=ident, in_=ident_dram.ap())
        wq_st = pp.tile([P, 6, C], F32)
        nc.scalar.dma_start(out=wq_st, in_=wqkv.rearrange("(a p) c -> p a c", p=P))
        wp_st = pp.tile([P, 2, C], F32)
        nc.scalar.dma_start(out=wp_st, in_=wproj.rearrange("(a p) c -> p a c", p=P))
        # selg[p, g] = 1 iff p//8 == g  (sums 8 consecutive partitions)
        selg = pp.tile([P, 16], F32)
        nc.scalar.dma_start(out=selg, in_=selg_dram.ap())
        # selgT[g, p] = 1 iff p//8 == g  (broadcasts group value to 8 partitions)
        selgT = pp.tile([16, P], F32)
        nc.scalar.dma_start(out=selgT, in_=selgT_dram.ap())

        eps16 = pp.tile([16, 1], F32)
        nc.vector.memset(eps16, EPS)
        ones64 = pp.tile([1, 64], F32)
        nc.vector.memset(ones64, 1.0)
        # Dummy Ln at t~0 pulls the natural_log_exp_and_others table load off
        # the critical path; Exp lives in the same set, so no later switch.
        scr16 = pp.tile([16, 1], F32)
        nc.scalar.activation(out=scr16, in_=eps16, func=AF.Ln, bias=eps16, scale=1.0)

        # ---------------- input DMAs ----------------
        # x: 4 MB, bus-bound. Split issue across SP + Pool queues so the
        # descriptor-generation overhead pipelines.
        x_sb = [pp.tile([P, T], F32, name=f"x_sb{ct}") for ct in range(2)]
        for ct in range(2):
            eng = nc.sync if ct == 0 else nc.gpsimd
            for chk in range(4):
                eng.dma_start(
                    out=x_sb[ct][:, 1024 * chk:1024 * (chk + 1)],
                    in_=xp[P * ct:P * (ct + 1), 1024 * chk:1024 * (chk + 1)])

        bq = [pp.tile([P, 1], F32, name=f"bq{p_}") for p_ in range(2)]
        bk = [pp.tile([P, 1], F32, name=f"bk{p_}") for p_ in range(2)]
        bv = [pp.tile([P, 1], F32, name=f"bv{p_}") for p_ in range(2)]
        for pr in range(2):
            for a in range(2):
                h = 2 * pr + a
                sl = slice(64 * a, 64 * (a + 1))
                nc.scalar.dma_start(out=bq[pr][sl, :], in_=bqkv[192 * h:192 * h + 64, :])
                nc.scalar.dma_start(out=bk[pr][sl, :], in_=bqkv[192 * h + 64:192 * h + 128, :])
                nc.scalar.dma_start(out=bv[pr][sl, :], in_=bqkv[192 * h + 128:192 * h + 192, :])
        gam = [pp.tile([P, 1], F32, name=f"gam{ct}") for ct in range(2)]
        bet = [pp.tile([P, 1], F32, name=f"bet{ct}") for ct in range(2)]
        bp = [pp.tile([P, 1], F32, name=f"bp{ct}") for ct in range(2)]
        for ct in range(2):
            sl = slice(P * ct, P * (ct + 1))
            nc.scalar.dma_start(out=gam[ct], in_=gamma[sl, :])
            nc.scalar.dma_start(out=bet[ct], in_=beta[sl, :])
            nc.scalar.dma_start(out=bp[ct], in_=bproj[sl, :])

        # ---------------- weight transposes (PE identity matmuls) ----------
        # Emitted before GroupNorm: they only depend on the weight DMAs, so
        # they fill the otherwise-idle PE during the x transfer.
        wq_bf = pp.tile([P, 6, C], BF16)
        nc.vector.tensor_copy(out=wq_bf, in_=wq_st)
        wp_bf = pp.tile([P, 2, C], BF16)
        nc.vector.tensor_copy(out=wp_bf, in_=wp_st)
        # WTq column layout: [q_p0 | q_p1 | k_p0 | k_p1 | v_p0 | v_p1], each a
        # contiguous 128-col block (pair = heads 2p,2p+1), so matmul operand
        # slices are single-free-dim APs.
        WTq = [pp.tile([P, 768], BF16, name=f"WTq{j}") for j in range(2)]
        WTp = [pp.tile([P, C], BF16, name=f"WTp{j}") for j in range(2)]
        for i in range(6):
            for j in range(2):
                tq_ps = ps.tile([P, P], BF16, tag="st", name=f"tq_ps{i}{j}")
                nc.tensor.transpose(out=tq_ps, in_=wq_bf[:, i, P * j:P * (j + 1)],
                                    identity=ident)
                for a in range(2):
                    o0 = P * i + 64 * a
                    h = o0 // 192
                    kind = (o0 % 192) // 64
                    dcol = kind * 256 + (h // 2) * 128 + (h % 2) * 64
                    nc.vector.tensor_copy(out=WTq[j][:, dcol:dcol + 64],
                                          in_=tq_ps[:, 64 * a:64 * (a + 1)])
        for i in range(2):
            for j in range(2):
                tp_ps = ps.tile([P, P], BF16, tag="st", name=f"tp_ps{i}{j}")
                nc.tensor.transpose(out=tp_ps, in_=wp_bf[:, i, P * j:P * (j + 1)],
                                    identity=ident)
                nc.vector.tensor_copy(out=WTp[j][:, P * i:P * (i + 1)], in_=tp_ps)

        def wt_slice(j, kind, pr):
            base = kind * 256 + pr * 128
            return WTq[j][:, base:base + 128]

        # ---------------- GroupNorm statistics ----------------
        # Per-channel mean/var via bn_stats, then 8-channel group sums via a
        # tiny f32 selector matmul, then broadcast back the same way.
        rhs_stats = pp.tile([P, 4], F32)
        for ct in range(2):
            xv = x_sb[ct].rearrange("p (n f) -> p n f", f=512)
            stats = wk.tile([P, 8, 6], F32, tag="bnstats")
            for i in range(8):
                nc.vector.bn_stats(out=stats[:, i, :], in_=xv[:, i, :])
            mv = wk.tile([P, 2], F32, tag="bnmv")
            nc.vector.bn_aggr(out=mv, in_=stats)
            nc.vector.tensor_copy(out=rhs_stats[:, 2 * ct:2 * ct + 1], in_=mv[:, 0:1])
            m2 = wk.tile([P, 1], F32, tag="m2")
            nc.vector.tensor_mul(out=m2, in0=mv[:, 0:1], in1=mv[:, 0:1])
            nc.vector.tensor_add(out=rhs_stats[:, 2 * ct + 1:2 * ct + 2],
                                 in0=mv[:, 1:2], in1=m2)

        gst_ps = ps.tile([16, 4], F32, tag="st")
        nc.tensor.matmul(out=gst_ps, lhsT=selg, rhs=rhs_stats, start=True, stop=True)
        gstat = pp.tile([16, 4], F32)
        nc.vector.tensor_scalar_mul(out=gstat, in0=gst_ps, scalar1=1.0 / GS)
        g3 = gstat.rearrange("p (a b) -> p a b", b=2)
        mu2 = pp.tile([16, 2], F32)
        nc.vector.tensor_mul(out=mu2, in0=g3[:, :, 0], in1=g3[:, :, 0])
        var_t = pp.tile([16, 2], F32)
        nc.vector.tensor_sub(out=var_t, in0=g3[:, :, 1], in1=mu2)
        # rstd = exp(-0.5*ln(var+eps)): Ln and Exp share one ACT table set
        # (natural_log_exp_and_others), so no mid-kernel table switch; also
        # drops the slow DVE reciprocal.
        ln_t = pp.tile([16, 2], F32)
        nc.scalar.activation(out=ln_t, in_=var_t, func=AF.Ln, bias=eps16, scale=1.0)
        rs_t = pp.tile([16, 2], F32)
        nc.scalar.activation(out=rs_t, in_=ln_t, func=AF.Exp, scale=-0.5)

        A_t = [pp.tile([P, 1], F32, name=f"A_t{ct}") for ct in range(2)]
        B_t = [pp.tile([P, 1], F32, name=f"B_t{ct}") for ct in range(2)]
        for ct in range(2):
            rhs_bc = wk.tile([16, 2], F32, tag="rhsbc")
            nc.vector.tensor_copy(out=rhs_bc[:, 0:1], in_=gstat[:, 2 * ct:2 * ct + 1])
            nc.vector.tensor_copy(out=rhs_bc[:, 1:2], in_=rs_t[:, ct:ct + 1])
            bc_ps = ps.tile([P, 2], F32, tag="st", name=f"bc_ps{ct}")
            nc.tensor.matmul(out=bc_ps, lhsT=selgT, rhs=rhs_bc, start=True, stop=True)
            nc.vector.tensor_mul(out=A_t[ct], in0=bc_ps[:, 1:2], in1=gam[ct])
            tb = wk.tile([P, 1], F32, tag="tb")
            nc.vector.tensor_mul(out=tb, in0=bc_ps[:, 0:1], in1=A_t[ct])
            nc.vector.tensor_sub(out=B_t[ct], in0=bet[ct], in1=tb)

        # h = x * A + B   (A = rstd*gamma, B = beta - mu*rstd*gamma), cast bf16
        h_bf = [pp.tile([P, T], BF16, name=f"h_bf{ct}") for ct in range(2)]
        for chk in range(4):
            csl = slice(1024 * chk, 1024 * (chk + 1))
            for ct in range(2):
                nc.vector.tensor_scalar(out=h_bf[ct][:, csl], in0=x_sb[ct][:, csl],
                                        scalar1=A_t[ct], scalar2=B_t[ct],
                                        op0=OP.mult, op1=OP.add)

        # residual + proj bias staged into the output buffer
        out_sb = [pp.tile([P, TQ], F32, name=f"out_sb{ct}") for ct in range(2)]
        for ct in range(2):
            nc.vector.tensor_scalar_add(out=out_sb[ct], in0=x_sb[ct][:, 0:TQ],
                                        scalar1=bp[ct])

        # ---------------- QKV projections ----------------
        q_sb = [pp.tile([P, TQ], BF16, name=f"q_sb{p_}") for p_ in range(2)]
        k_sb = [pp.tile([P, T], BF16, name=f"k_sb{p_}") for p_ in range(2)]
        vT_sb = [pp.tile([P, 32, 2, 65], BF16, name=f"vT_sb{p_}") for p_ in range(2)]

        def gen_qkv_stripe(pr, tt):
            # Two-part stripe emission (k+q, then v) so slot demand on the
            # shared "st" psum tag spreads across attention pipeline steps.
            it_ = iter(_qkv_stripe_parts(pr, tt))
            return it_

        def emit_qkv_stripe(pr, tt):
            for _ in _qkv_stripe_parts(pr, tt):
                pass

        def _qkv_stripe_parts(pr, tt):
            # One 512-column stripe: k (+q for tt<4) and vT for pair pr.
            # k/q share one st-tagged psum tile (2 banks) to limit slot churn
            # while interleaved with the attention pipeline.
            if tt == 0:
                nc.vector.memset(vT_sb[pr][:, :, :, 64:65], 1.0)
            tsl = slice(512 * tt, 512 * (tt + 1))
            s1 = ps.tile([P, 2, 512], F32, tag="st", name=f"qk_ps{pr}{tt}")
            for ct in range(2):
                nc.tensor.matmul(out=s1[:, 0, :], lhsT=wt_slice(ct, 1, pr),
                                 rhs=h_bf[ct][:, tsl],
                                 start=(ct == 0), stop=(ct == 1))
            nc.vector.tensor_scalar_add(out=k_sb[pr][:, tsl], in0=s1[:, 0, :],
                                        scalar1=bk[pr])
            if tt < 4:
                for ct in range(2):
                    nc.tensor.matmul(out=s1[:, 1, :], lhsT=wt_slice(ct, 0, pr),
                                     rhs=h_bf[ct][:, tsl],
                                     start=(ct == 0), stop=(ct == 1))
                nc.vector.tensor_scalar_add(out=q_sb[pr][:, tsl],
                                            in0=s1[:, 1, :], scalar1=bq[pr])
                yield
                vtile = ps.tile([P, 2, 512], F32, tag="st", name=f"v_ps{pr}{tt}")
                vsl = vtile[:, 0, :]
            else:
                yield
                vsl = s1[:, 1, :]
            vv = vsl.rearrange("p (j n) -> p j n", j=4)
            for j in range(4):
                it = 4 * tt + j
                for ct in range(2):
                    nc.tensor.matmul(out=vv[:, j, :],
                                     lhsT=h_bf[ct][:, P * it:P * (it + 1)],
                                     rhs=wt_slice(ct, 2, pr),
                                     start=(ct == 0), stop=(ct == 1))
            nc.vector.tensor_copy(
                out=vT_sb[pr][:, 4 * tt:4 * (tt + 1), :, 0:64],
                in_=vsl.rearrange("p (j h c) -> p j h c", j=4, h=2))
            yield

        # ---------------- attention ----------------
        # KSTAGE: debug knob — 2 = stop after qkv, 3 = one attention tile only.
        kstage = int(os.environ.get("KSTAGE", "9"))
        a_sb = [pp.tile([P, TQ], BF16, name=f"a_sb{p_}") for p_ in range(2)]
        if kstage <= 3:
            for pr_ in range(2):
                nc.vector.memset(a_sb[pr_], 0.0)
        n_tt = 0 if kstage <= 2 else (1 if kstage == 3 else 4)

        gens_pv = {}

        def gen_attn(pr, tt):
            """Pipeline-step generator for one (pair, q-tile): step s emits the
            two ST matmuls + exp for key-block s (s<32) and the PV matmuls for
            block s-1 — ST_{s+1} always precedes PV_s in the in-order PE stream
            so ACT (the bottleneck) never starves."""
            tsl = slice(512 * tt, 512 * (tt + 1))
            pv = [ps.tile([65, 512], F32, tag="pv", bufs=3,
                          name=f"pv{pr}{tt}{h}") for h in range(2)]
            gens_pv[(pr, tt)] = pv
            pts = {}
            for s in range(33):
                if s < 32:
                    st = ps.tile([P, 2, 512], F32, tag="st", bufs=2,
                                 name=f"st{pr}{tt}{s}")
                    for h in range(2):
                        nc.tensor.matmul(
                            out=st[:, h, :],
                            lhsT=k_sb[pr][64 * h:64 * (h + 1), P * s:P * (s + 1)],
                            rhs=q_sb[pr][64 * h:64 * (h + 1), tsl],
                            start=True, stop=True)
                    pt = ptp.tile([P, 2, 512], BF16, tag="pt",
                                  name=f"pt{pr}{tt}{s}")
                    nc.scalar.activation(out=pt, in_=st, func=AF.Exp, scale=SCALE)
                    pts[s] = pt
                if s >= 1:
                    pt_prev = pts.pop(s - 1)
                    for h in range(2):
                        nc.tensor.matmul(out=pv[h],
                                         lhsT=vT_sb[pr][:, s - 1, h, :],
                                         rhs=pt_prev[:, h, :],
                                         start=(s == 1), stop=(s == 32))
                yield

        def emit_normalize(pr, tt):
            # a = pv[0:64] / Z + v_bias ; Z sits in row 64. 1/Z broadcast to
            # 64 partitions via a K=1 f32 ones-matmul.
            pv = gens_pv.pop((pr, tt))
            tsl = slice(512 * tt, 512 * (tt + 1))
            for h in range(2):
                rz = wk.tile([1, 512], F32, tag="rz")
                nc.vector.reciprocal(out=rz, in_=pv[h][64:65, :])
                rzb_ps = ps.tile([64, 512], F32, tag="bc", bufs=1,
                                 name=f"rzb_ps{pr}{tt}{h}")
                nc.tensor.matmul(out=rzb_ps, lhsT=ones64, rhs=rz,
                                 start=True, stop=True)
                rzb = wk.tile([64, 512], F32, tag="rzb")
                nc.vector.tensor_copy(out=rzb, in_=rzb_ps)
                asl = a_sb[pr][64 * h:64 * (h + 1), tsl]
                nc.vector.tensor_mul(out=asl, in0=pv[h][0:64, :], in1=rzb)
                nc.vector.tensor_scalar_add(out=asl, in0=asl,
                                            scalar1=bv[pr][64 * h:64 * (h + 1), :])

        proj_done = set()

        def emit_proj_tt(tt):
            # proj + residual for one 512-column stripe (both output chunks);
            # psums borrow the "bc" tag, idle outside normalize.
            proj_done.add(tt)
            tsl = slice(512 * tt, 512 * (tt + 1))
            for oc in range(2):
                pj = ps.tile([P, 512], F32, tag="bc", bufs=1, name=f"pj{oc}{tt}")
                for ct in range(2):
                    nc.tensor.matmul(out=pj, lhsT=WTp[ct][:, P * oc:P * (oc + 1)],
                                     rhs=a_sb[ct][:, tsl],
                                     start=(ct == 0), stop=(ct == 1))
                nc.vector.tensor_add(out=out_sb[oc][:, tsl],
                                     in0=out_sb[oc][:, tsl], in1=pj)
                nc.sync.dma_start(out=outp[P * oc:P * (oc + 1), tsl],
                                  in_=out_sb[oc][:, tsl])

        def drive(g, n):
            for _ in range(n):
                next(g, None)

        # ---- flat schedule: qkv stripes interleaved into attention steps ----
        emit_qkv_stripe(0, 0)
        tiles = [(0, t_) for t_ in range(n_tt)]
        if kstage >= 4:
            tiles += [(1, t_) for t_ in range(n_tt)]
        prev = None
        for idx, (pr, tt) in enumerate(tiles):
            g = gen_attn(pr, tt)
            if idx == 0:
                # remaining p0 stripes, split k+q / v across each 4-step
                # window; ST block s only needs stripes up to s//4, already
                # fully emitted by the window boundary.
                for i in range(1, 8):
                    sp = gen_qkv_stripe(0, i)
                    drive(g, 2)
                    next(sp, None)
                    drive(g, 2)
                    next(sp, None)
                    next(sp, None)
                drive(g, 5)
            else:
                drive(g, 2)          # ST0/exp0/ST1/exp1/PV0 queued for ACT
                if prev is not None:
                    emit_normalize(*prev)
                drive(g, 1)
                if prev is not None and prev[0] == 1:
                    emit_proj_tt(prev[1])
                if kstage >= 4 and (pr, tt) == (0, 3):
                    done = 3         # p1 stripes hidden under tile (0,3)
                    for si in range(8):
                        sp = gen_qkv_stripe(1, si)
                        mid = min(4 * si + 2, 33)
                        drive(g, max(0, mid - done))
                        done = mid
                        next(sp, None)
                        target = min(4 * (si + 1), 33)
                        drive(g, max(0, target - done))
                        done = target
                        next(sp, None)
                        next(sp, None)
                    drive(g, 33 - done)
                else:
                    drive(g, 30)
            prev = (pr, tt)
        if prev is not None:
            emit_normalize(*prev)
        if kstage >= 4 and kstage < 9:
            pass
        if kstage < 4:
            emit_qkv_stripe(1, 0)    # keep p1 buffers initialized
        for t_ in range(4):
            if t_ not in proj_done:
                emit_proj_tt(t_)


@functools.cache
def _build():
    nc = bacc.Bacc("TRN2", target_bir_lowering=False, debug=False)
    xp = nc.dram_tensor("xp", [C, T], F32, kind="ExternalInput").ap()
    wqkv = nc.dram_tensor("wqkv", [3 * C, C], F32, kind="ExternalInput").ap()
    bqkv = nc.dram_tensor("bqkv", [3 * C, 1], F32, kind="ExternalInput").ap()
    wproj = nc.dram_tensor("wproj", [C, C], F32, kind="ExternalInput").ap()
    bproj = nc.dram_tensor("bproj", [C, 1], F32, kind="ExternalInput").ap()
    gamma = nc.dram_tensor("gamma", [C, 1], F32, kind="ExternalInput").ap()
    beta = nc.dram_tensor("beta", [C, 1], F32, kind="ExternalInput").ap()
    outp = nc.dram_tensor("outp", [C, TQ], F32, kind="ExternalOutput").ap()
    with tile.TileContext(nc) as tc:
        _emit(nc, tc, (xp, wqkv, bqkv, wproj, bproj, gamma, beta, outp))
    nc.finalize()
    return nc


def _make_in_maps(x, gamma, beta, w_qkv, qkv_bias, w_proj, proj_bias):
    xf = np.ascontiguousarray(np.asarray(x, np.float32)).reshape(4, C, T)
    shared = {
        "wqkv": np.ascontiguousarray(np.asarray(w_qkv, np.float32)),
        "bqkv": np.ascontiguousarray(np.asarray(qkv_bias, np.float32).reshape(3 * C, 1)),
        "wproj": np.ascontiguousarray(np.asarray(w_proj, np.float32)),
        "bproj": np.ascontiguousarray(np.asarray(proj_bias, np.float32).reshape(C, 1)),
        "gamma": np.ascontiguousarray(np.asarray(gamma, np.float32).reshape(C, 1)),
        "beta": np.ascontiguousarray(np.asarray(beta, np.float32).reshape(C, 1)),
    }
    in_maps = []
    for core in range(N_CORES):
        b, half = divmod(core, 2)
        if half == 0:
            xpc = xf[b]
        else:
            xpc = np.concatenate([xf[b][:, TQ:], xf[b][:, :TQ]], axis=1)
        in_maps.append({"xp": np.ascontiguousarray(xpc), **shared})
    return in_maps


def _run(in_maps, **kwargs):
    nc = _build()
    return run_bass_kernel_spmd(nc, in_maps, core_ids=list(range(N_CORES)), **kwargs)


def kernel(x, gamma, beta, w_qkv, qkv_bias, w_proj, proj_bias, num_heads):
    assert int(num_heads) == 4
    in_maps = _make_in_maps(x, gamma, beta, w_qkv, qkv_bias, w_proj, proj_bias)
    res = _run(in_maps)
    out = np.empty((4, C, T), np.float32)
    for core in range(N_CORES):
        b, half = divmod(core, 2)
        out[b][:, half * TQ:(half + 1) * TQ] = res.results[core]["outp"]
    return out.reshape(4, C, 64, 64)



# revision 7
# speedup vs baseline: 1.0414x; 1.0159x over previous
"""Trainium2 Bass kernel for nn_AttentionBlock (GroupNorm + 4-head attention + proj + residual).

Problem (hardcoded): x [4, 256, 64, 64] f32, 32 groups, 4 heads (ch=64/head),
T = 64*64 = 4096 tokens per batch item.

Sharding over 8 NeuronCores: core = (batch b, query-half). Each core receives
x[b] with its token-half rotated to the front (attention is invariant to a
consistent permutation of key/value order, and queries are sliced), computes
GroupNorm(x[b]) + full K/V for all 4 heads, flash-attention for its 2048
queries, proj + residual for those tokens. Outputs are disjoint -> the host
just stitches halves back together. No collectives needed.

Inside a core everything is bf16 on the PE (f32 PSUM accumulation); softmax
runs unnormalized (exp on ScalarE, row-sums via an appended ones-column in
the P@V matmul, normalization folded in afterwards on VectorE).
"""

import functools
import os

import numpy as np

import concourse.bass as bass
import concourse.tile as tile
from concourse import bacc, mybir
from concourse.bass_utils import run_bass_kernel_spmd

F32 = mybir.dt.float32
BF16 = mybir.dt.bfloat16
AF = mybir.ActivationFunctionType
OP = mybir.AluOpType

P = 128          # partitions
C = 256          # channels
T = 4096         # tokens per batch item
TQ = 2048        # query tokens per core (half of T)
CH = 64          # channels per head
GS = 8           # channels per group
EPS = 1e-5
SCALE = 0.125    # 1/sqrt(ch) applied inside exp
N_CORES = 8


def _emit(nc, tc, aps):
    xp, wqkv, bqkv, wproj, bproj, gamma, beta, outp = aps

    with (
        tc.tile_pool(name="pp", bufs=1) as pp,
        tc.tile_pool(name="wk", bufs=2) as wk,
        tc.tile_pool(name="ptp", bufs=6) as ptp,
        tc.tile_pool(name="ps", bufs=2, space="PSUM") as ps,
    ):
        # ---------------- constants (NEFF-embedded, no gpsimd) ----------------
        import ml_dtypes
        ident_np = np.eye(P, dtype=ml_dtypes.bfloat16)
        selg_np = (np.arange(P)[:, None] // GS == np.arange(16)[None, :]).astype(np.float32)
        ident_dram = nc.inline_tensor(ident_np, name="ident_c")
        selg_dram = nc.inline_tensor(selg_np, name="selg_c")
        selgT_dram = nc.inline_tensor(np.ascontiguousarray(selg_np.T), name="selgT_c")

        # Weights/consts ride the ACT queue so they land ~t=0 (the big x
        # transfers own the SP queue); PE transposes then start immediately.
        ident = pp.tile([P, P], BF16)
        nc.scalar.dma_start(out=ident, in_=ident_dram.ap())
        wq_st = pp.tile([P, 6, C], F32)
        nc.scalar.dma_start(out=wq_st, in_=wqkv.rearrange("(a p) c -> p a c", p=P))
        wp_st = pp.tile([P, 2, C], F32)
        nc.scalar.dma_start(out=wp_st, in_=wproj.rearrange("(a p) c -> p a c", p=P))
        # selg[p, g] = 1 iff p//8 == g  (sums 8 consecutive partitions)
        selg = pp.tile([P, 16], F32)
        nc.scalar.dma_start(out=selg, in_=selg_dram.ap())
        # selgT[g, p] = 1 iff p//8 == g  (broadcasts group value to 8 partitions)
        selgT = pp.tile([16, P], F32)
        nc.scalar.dma_start(out=selgT, in_=selgT_dram.ap())

        eps16 = pp.tile([16, 1], F32)
        nc.vector.memset(eps16, EPS)
        ones64 = pp.tile([1, 64], F32)
        nc.vector.memset(ones64, 1.0)
        # Dummy Ln at t~0 pulls the natural_log_exp_and_others table load off
        # the critical path; Exp lives in the same set, so no later switch.
        scr16 = pp.tile([16, 1], F32)
        nc.scalar.activation(out=scr16, in_=eps16, func=AF.Ln, bias=eps16, scale=1.0)

        # ---------------- input DMAs ----------------
        # x: 4 MB, bus-bound. Split issue across SP + Pool queues so the
        # descriptor-generation overhead pipelines.
        x_sb = [pp.tile([P, T], F32, name=f"x_sb{ct}") for ct in range(2)]
        for ct in range(2):
            eng = nc.sync if ct == 0 else nc.gpsimd
            for chk in range(4):
                eng.dma_start(
                    out=x_sb[ct][:, 1024 * chk:1024 * (chk + 1)],
                    in_=xp[P * ct:P * (ct + 1), 1024 * chk:1024 * (chk + 1)])

        # Biases ride the Pool queue (cheap issue; Pool is idle after its x
        # chunks). gam/bet are needed first (GroupNorm A/B at ~10 us).
        bq = [pp.tile([P, 1], F32, name=f"bq{p_}") for p_ in range(2)]
        bk = [pp.tile([P, 1], F32, name=f"bk{p_}") for p_ in range(2)]
        bv = [pp.tile([P, 1], F32, name=f"bv{p_}") for p_ in range(2)]
        gam = [pp.tile([P, 1], F32, name=f"gam{ct}") for ct in range(2)]
        bet = [pp.tile([P, 1], F32, name=f"bet{ct}") for ct in range(2)]
        bp = [pp.tile([P, 1], F32, name=f"bp{ct}") for ct in range(2)]
        for ct in range(2):
            sl = slice(P * ct, P * (ct + 1))
            nc.gpsimd.dma_start(out=gam[ct], in_=gamma[sl, :])
            nc.gpsimd.dma_start(out=bet[ct], in_=beta[sl, :])
            nc.gpsimd.dma_start(out=bp[ct], in_=bproj[sl, :])
        for pr in range(2):
            for a in range(2):
                h = 2 * pr + a
                sl = slice(64 * a, 64 * (a + 1))
                nc.gpsimd.dma_start(out=bq[pr][sl, :], in_=bqkv[192 * h:192 * h + 64, :])
                nc.gpsimd.dma_start(out=bk[pr][sl, :], in_=bqkv[192 * h + 64:192 * h + 128, :])
                nc.gpsimd.dma_start(out=bv[pr][sl, :], in_=bqkv[192 * h + 128:192 * h + 192, :])
        # ---------------- GroupNorm statistics ----------------
        # Per-channel mean/var via bn_stats, then 8-channel group sums via a
        # tiny f32 selector matmul, then broadcast back the same way.
        rhs_stats = pp.tile([P, 4], F32)
        for ct in range(2):
            xv = x_sb[ct].rearrange("p (n f) -> p n f", f=512)
            stats = wk.tile([P, 8, 6], F32, tag="bnstats")
            for i in range(8):
                nc.vector.bn_stats(out=stats[:, i, :], in_=xv[:, i, :])
            mv = wk.tile([P, 2], F32, tag="bnmv")
            nc.vector.bn_aggr(out=mv, in_=stats)
            nc.vector.tensor_copy(out=rhs_stats[:, 2 * ct:2 * ct + 1], in_=mv[:, 0:1])
            m2 = wk.tile([P, 1], F32, tag="m2")
            nc.vector.tensor_mul(out=m2, in0=mv[:, 0:1], in1=mv[:, 0:1])
            nc.vector.tensor_add(out=rhs_stats[:, 2 * ct + 1:2 * ct + 2],
                                 in0=mv[:, 1:2], in1=m2)

        gst_ps = ps.tile([16, 4], F32, tag="st")
        nc.tensor.matmul(out=gst_ps, lhsT=selg, rhs=rhs_stats, start=True, stop=True)
        gstat = pp.tile([16, 4], F32)
        nc.vector.tensor_scalar_mul(out=gstat, in0=gst_ps, scalar1=1.0 / GS)
        g3 = gstat.rearrange("p (a b) -> p a b", b=2)
        mu2 = pp.tile([16, 2], F32)
        nc.vector.tensor_mul(out=mu2, in0=g3[:, :, 0], in1=g3[:, :, 0])
        var_t = pp.tile([16, 2], F32)
        nc.vector.tensor_sub(out=var_t, in0=g3[:, :, 1], in1=mu2)
        # rstd = exp(-0.5*ln(var+eps)): Ln and Exp share one ACT table set
        # (natural_log_exp_and_others), so no mid-kernel table switch; also
        # drops the slow DVE reciprocal.
        ln_t = pp.tile([16, 2], F32)
        nc.scalar.activation(out=ln_t, in_=var_t, func=AF.Ln, bias=eps16, scale=1.0)
        rs_t = pp.tile([16, 2], F32)
        nc.scalar.activation(out=rs_t, in_=ln_t, func=AF.Exp, scale=-0.5)

        A_t = [pp.tile([P, 1], F32, name=f"A_t{ct}") for ct in range(2)]
        B_t = [pp.tile([P, 1], F32, name=f"B_t{ct}") for ct in range(2)]
        for ct in range(2):
            rhs_bc = wk.tile([16, 2], F32, tag="rhsbc")
            nc.vector.tensor_copy(out=rhs_bc[:, 0:1], in_=gstat[:, 2 * ct:2 * ct + 1])
            nc.vector.tensor_copy(out=rhs_bc[:, 1:2], in_=rs_t[:, ct:ct + 1])
            bc_ps = ps.tile([P, 2], F32, tag="st", name=f"bc_ps{ct}")
            nc.tensor.matmul(out=bc_ps, lhsT=selgT, rhs=rhs_bc, start=True, stop=True)
            nc.vector.tensor_mul(out=A_t[ct], in0=bc_ps[:, 1:2], in1=gam[ct])
            tb = wk.tile([P, 1], F32, tag="tb")
            nc.vector.tensor_mul(out=tb, in0=bc_ps[:, 0:1], in1=A_t[ct])
            nc.vector.tensor_sub(out=B_t[ct], in0=bet[ct], in1=tb)

        # h = x * A + B   (A = rstd*gamma, B = beta - mu*rstd*gamma), cast bf16
        # ct0 on DVE, ct1 on GPSIMD: halves the DVE serial work on the ramp's
        # critical path (stats -> A/B -> h -> first stripe).
        h_bf = [pp.tile([P, T], BF16, name=f"h_bf{ct}") for ct in range(2)]
        for chk in range(4):
            csl = slice(1024 * chk, 1024 * (chk + 1))
            for ct in range(2):
                eng = nc.vector if ct == 0 else nc.gpsimd
                eng.tensor_scalar(out=h_bf[ct][:, csl], in0=x_sb[ct][:, csl],
                                  scalar1=A_t[ct], scalar2=B_t[ct],
                                  op0=OP.mult, op1=OP.add)

        # ---------------- weight transposes (PE identity matmuls) ----------
        # Emitted after the GN chain so their DVE copies don't preempt the
        # ramp-critical stats/A/B work; the PE runs them during the x DMA.
        wq_bf = pp.tile([P, 6, C], BF16)
        nc.vector.tensor_copy(out=wq_bf, in_=wq_st)
        wp_bf = pp.tile([P, 2, C], BF16)
        nc.vector.tensor_copy(out=wp_bf, in_=wp_st)
        # WTq column layout: [q_p0 | q_p1 | k_p0 | k_p1 | v_p0 | v_p1], each a
        # contiguous 128-col block (pair = heads 2p,2p+1), so matmul operand
        # slices are single-free-dim APs.
        WTq = [pp.tile([P, 768], BF16, name=f"WTq{j}") for j in range(2)]
        WTp = [pp.tile([P, C], BF16, name=f"WTp{j}") for j in range(2)]
        for i in range(6):
            for j in range(2):
                tq_ps = ps.tile([P, P], BF16, tag="st", name=f"tq_ps{i}{j}")
                nc.tensor.transpose(out=tq_ps, in_=wq_bf[:, i, P * j:P * (j + 1)],
                                    identity=ident)
                for a in range(2):
                    o0 = P * i + 64 * a
                    h = o0 // 192
                    kind = (o0 % 192) // 64
                    dcol = kind * 256 + (h // 2) * 128 + (h % 2) * 64
                    nc.vector.tensor_copy(out=WTq[j][:, dcol:dcol + 64],
                                          in_=tq_ps[:, 64 * a:64 * (a + 1)])
        for i in range(2):
            for j in range(2):
                tp_ps = ps.tile([P, P], BF16, tag="st", name=f"tp_ps{i}{j}")
                nc.tensor.transpose(out=tp_ps, in_=wp_bf[:, i, P * j:P * (j + 1)],
                                    identity=ident)
                nc.vector.tensor_copy(out=WTp[j][:, P * i:P * (i + 1)], in_=tp_ps)

        def wt_slice(j, kind, pr):
            base = kind * 256 + pr * 128
            return WTq[j][:, base:base + 128]

        # residual + proj bias staged into the output buffer (GPSIMD: not
        # ramp-critical, only needed by proj mid-kernel)
        out_sb = [pp.tile([P, TQ], F32, name=f"out_sb{ct}") for ct in range(2)]
        for ct in range(2):
            nc.gpsimd.tensor_scalar_add(out=out_sb[ct], in0=x_sb[ct][:, 0:TQ],
                                        scalar1=bp[ct])

        # ---------------- QKV projections ----------------
        q_sb = [pp.tile([P, TQ], BF16, name=f"q_sb{p_}") for p_ in range(2)]
        k_sb = [pp.tile([P, T], BF16, name=f"k_sb{p_}") for p_ in range(2)]
        vT_sb = [pp.tile([P, 32, 2, 65], BF16, name=f"vT_sb{p_}") for p_ in range(2)]

        def gen_qkv_stripe(pr, tt):
            # Two-part stripe emission (k+q, then v) so slot demand on the
            # shared "st" psum tag spreads across attention pipeline steps.
            it_ = iter(_qkv_stripe_parts(pr, tt))
            return it_

        def emit_qkv_stripe(pr, tt):
            for _ in _qkv_stripe_parts(pr, tt):
                pass

        def _qkv_stripe_parts(pr, tt):
            # One 512-column stripe: k (+q for tt<4) and vT for pair pr.
            # k/q share one st-tagged psum tile (2 banks) to limit slot churn
            # while interleaved with the attention pipeline.
            if tt == 0:
                nc.vector.memset(vT_sb[pr][:, :, :, 64:65], 1.0)
            tsl = slice(512 * tt, 512 * (tt + 1))
            s1 = ps.tile([P, 2, 512], F32, tag="st", name=f"qk_ps{pr}{tt}")
            for ct in range(2):
                nc.tensor.matmul(out=s1[:, 0, :], lhsT=wt_slice(ct, 1, pr),
                                 rhs=h_bf[ct][:, tsl],
                                 start=(ct == 0), stop=(ct == 1))
            nc.vector.tensor_scalar_add(out=k_sb[pr][:, tsl], in0=s1[:, 0, :],
                                        scalar1=bk[pr])
            if tt < 4:
                for ct in range(2):
                    nc.tensor.matmul(out=s1[:, 1, :], lhsT=wt_slice(ct, 0, pr),
                                     rhs=h_bf[ct][:, tsl],
                                     start=(ct == 0), stop=(ct == 1))
                nc.vector.tensor_scalar_add(out=q_sb[pr][:, tsl],
                                            in0=s1[:, 1, :], scalar1=bq[pr])
                yield
                vtile = ps.tile([P, 2, 512], F32, tag="st", name=f"v_ps{pr}{tt}")
                vsl = vtile[:, 0, :]
            else:
                yield
                vsl = s1[:, 1, :]
            vv = vsl.rearrange("p (j n) -> p j n", j=4)
            for j in range(4):
                it = 4 * tt + j
                for ct in range(2):
                    nc.tensor.matmul(out=vv[:, j, :],
                                     lhsT=h_bf[ct][:, P * it:P * (it + 1)],
                                     rhs=wt_slice(ct, 2, pr),
                                     start=(ct == 0), stop=(ct == 1))
            nc.vector.tensor_copy(
                out=vT_sb[pr][:, 4 * tt:4 * (tt + 1), :, 0:64],
                in_=vsl.rearrange("p (j h c) -> p j h c", j=4, h=2))
            yield

        # ---------------- attention ----------------
        # KSTAGE: debug knob — 2 = stop after qkv, 3 = one attention tile only.
        kstage = int(os.environ.get("KSTAGE", "9"))
        a_sb = [pp.tile([P, TQ], BF16, name=f"a_sb{p_}") for p_ in range(2)]
        if kstage <= 3:
            for pr_ in range(2):
                nc.vector.memset(a_sb[pr_], 0.0)
        n_tt = 0 if kstage <= 2 else (1 if kstage == 3 else 4)

        gens_pv = {}

        def gen_attn(pr, tt):
            """Pipeline-step generator for one (pair, q-tile): step s emits the
            two ST matmuls + exp for key-block s (s<32) and the PV matmuls for
            block s-1 — ST_{s+1} always precedes PV_s in the in-order PE stream
            so ACT (the bottleneck) never starves."""
            tsl = slice(512 * tt, 512 * (tt + 1))
            pv = [ps.tile([65, 512], F32, tag="pv", bufs=3,
                          name=f"pv{pr}{tt}{h}") for h in range(2)]
            gens_pv[(pr, tt)] = pv
            pts = {}
            for s in range(33):
                if s < 32:
                    st = ps.tile([P, 2, 512], F32, tag="st", bufs=2,
                                 name=f"st{pr}{tt}{s}")
                    for h in range(2):
                        nc.tensor.matmul(
                            out=st[:, h, :],
                            lhsT=k_sb[pr][64 * h:64 * (h + 1), P * s:P * (s + 1)],
                            rhs=q_sb[pr][64 * h:64 * (h + 1), tsl],
                            start=True, stop=True)
                    pt = ptp.tile([P, 2, 512], BF16, tag="pt",
                                  name=f"pt{pr}{tt}{s}")
                    nc.scalar.activation(out=pt, in_=st, func=AF.Exp, scale=SCALE)
                    pts[s] = pt
                if s >= 1:
                    pt_prev = pts.pop(s - 1)
                    for h in range(2):
                        nc.tensor.matmul(out=pv[h],
                                         lhsT=vT_sb[pr][:, s - 1, h, :],
                                         rhs=pt_prev[:, h, :],
                                         start=(s == 1), stop=(s == 32))
                yield

        def emit_normalize(pr, tt):
            # a = pv[0:64] / Z + v_bias ; Z sits in row 64. 1/Z broadcast to
            # 64 partitions via a K=1 f32 ones-matmul.
            pv = gens_pv.pop((pr, tt))
            tsl = slice(512 * tt, 512 * (tt + 1))
            for h in range(2):
                rz = wk.tile([1, 512], F32, tag="rz")
                nc.vector.reciprocal(out=rz, in_=pv[h][64:65, :])
                rzb_ps = ps.tile([64, 512], F32, tag="bc", bufs=1,
                                 name=f"rzb_ps{pr}{tt}{h}")
                nc.tensor.matmul(out=rzb_ps, lhsT=ones64, rhs=rz,
                                 start=True, stop=True)
                rzb = wk.tile([64, 512], F32, tag="rzb")
                nc.vector.tensor_copy(out=rzb, in_=rzb_ps)
                asl = a_sb[pr][64 * h:64 * (h + 1), tsl]
                nc.vector.tensor_mul(out=asl, in0=pv[h][0:64, :], in1=rzb)
                nc.vector.tensor_scalar_add(out=asl, in0=asl,
                                            scalar1=bv[pr][64 * h:64 * (h + 1), :])

        proj_done = set()

        def emit_proj_tt(tt):
            # proj + residual for one 512-column stripe (both output chunks);
            # psums borrow the "bc" tag, idle outside normalize.
            proj_done.add(tt)
            tsl = slice(512 * tt, 512 * (tt + 1))
            for oc in range(2):
                pj = ps.tile([P, 512], F32, tag="bc", bufs=1, name=f"pj{oc}{tt}")
                for ct in range(2):
                    nc.tensor.matmul(out=pj, lhsT=WTp[ct][:, P * oc:P * (oc + 1)],
                                     rhs=a_sb[ct][:, tsl],
                                     start=(ct == 0), stop=(ct == 1))
                nc.vector.tensor_add(out=out_sb[oc][:, tsl],
                                     in0=out_sb[oc][:, tsl], in1=pj)
                nc.sync.dma_start(out=outp[P * oc:P * (oc + 1), tsl],
                                  in_=out_sb[oc][:, tsl])

        def drive(g, n):
            for _ in range(n):
                next(g, None)

        # ---- flat schedule: qkv stripes interleaved into attention steps ----
        emit_qkv_stripe(0, 0)
        tiles = [(0, t_) for t_ in range(n_tt)]
        if kstage >= 4:
            tiles += [(1, t_) for t_ in range(n_tt)]
        prev = None
        for idx, (pr, tt) in enumerate(tiles):
            g = gen_attn(pr, tt)
            if idx == 0:
                # remaining p0 stripes, split k+q / v across each 4-step
                # window; ST block s only needs stripes up to s//4, already
                # fully emitted by the window boundary.
                for i in range(1, 8):
                    sp = gen_qkv_stripe(0, i)
                    drive(g, 2)
                    next(sp, None)
                    drive(g, 2)
                    next(sp, None)
                    next(sp, None)
                drive(g, 5)
            else:
                drive(g, 2)          # ST0/exp0/ST1/exp1/PV0 queued for ACT
                if prev is not None:
                    emit_normalize(*prev)
                drive(g, 1)
                if prev is not None and prev[0] == 1:
                    emit_proj_tt(prev[1])
                if kstage >= 4 and (pr, tt) == (0, 3):
                    done = 3         # p1 stripes hidden under tile (0,3)
                    for si in range(8):
                        sp = gen_qkv_stripe(1, si)
                        mid = min(4 * si + 2, 33)
                        drive(g, max(0, mid - done))
                        done = mid
                        next(sp, None)
                        target = min(4 * (si + 1), 33)
                        drive(g, max(0, target - done))
                        done = target
                        next(sp, None)
                        next(sp, None)
                    drive(g, 33 - done)
                else:
                    drive(g, 30)
            prev = (pr, tt)
        if prev is not None:
            emit_normalize(*prev)
        if kstage >= 4 and kstage < 9:
            pass
        if kstage < 4:
            emit_qkv_stripe(1, 0)    # keep p1 buffers initialized
        for t_ in range(4):
            if t_ not in proj_done:
                emit_proj_tt(t_)


@functools.cache
def _build():
    nc = bacc.Bacc("TRN2", target_bir_lowering=False, debug=False)
    xp = nc.dram_tensor("xp", [C, T], F32, kind="ExternalInput").ap()
    wqkv = nc.dram_tensor("wqkv", [3 * C, C], F32, kind="ExternalInput").ap()
    bqkv = nc.dram_tensor("bqkv", [3 * C, 1], F32, kind="ExternalInput").ap()
    wproj = nc.dram_tensor("wproj", [C, C], F32, kind="ExternalInput").ap()
    bproj = nc.dram_tensor("bproj", [C, 1], F32, kind="ExternalInput").ap()
    gamma = nc.dram_tensor("gamma", [C, 1], F32, kind="ExternalInput").ap()
    beta = nc.dram_tensor("beta", [C, 1], F32, kind="ExternalInput").ap()
    outp = nc.dram_tensor("outp", [C, TQ], F32, kind="ExternalOutput").ap()
    with tile.TileContext(nc) as tc:
        _emit(nc, tc, (xp, wqkv, bqkv, wproj, bproj, gamma, beta, outp))
    nc.finalize()
    return nc


def _make_in_maps(x, gamma, beta, w_qkv, qkv_bias, w_proj, proj_bias):
    xf = np.ascontiguousarray(np.asarray(x, np.float32)).reshape(4, C, T)
    shared = {
        "wqkv": np.ascontiguousarray(np.asarray(w_qkv, np.float32)),
        "bqkv": np.ascontiguousarray(np.asarray(qkv_bias, np.float32).reshape(3 * C, 1)),
        "wproj": np.ascontiguousarray(np.asarray(w_proj, np.float32)),
        "bproj": np.ascontiguousarray(np.asarray(proj_bias, np.float32).reshape(C, 1)),
        "gamma": np.ascontiguousarray(np.asarray(gamma, np.float32).reshape(C, 1)),
        "beta": np.ascontiguousarray(np.asarray(beta, np.float32).reshape(C, 1)),
    }
    in_maps = []
    for core in range(N_CORES):
        b, half = divmod(core, 2)
        if half == 0:
            xpc = xf[b]
        else:
            xpc = np.concatenate([xf[b][:, TQ:], xf[b][:, :TQ]], axis=1)
        in_maps.append({"xp": np.ascontiguousarray(xpc), **shared})
    return in_maps


def _run(in_maps, **kwargs):
    nc = _build()
    return run_bass_kernel_spmd(nc, in_maps, core_ids=list(range(N_CORES)), **kwargs)


def kernel(x, gamma, beta, w_qkv, qkv_bias, w_proj, proj_bias, num_heads):
    assert int(num_heads) == 4
    in_maps = _make_in_maps(x, gamma, beta, w_qkv, qkv_bias, w_proj, proj_bias)
    res = _run(in_maps)
    out = np.empty((4, C, T), np.float32)
    for core in range(N_CORES):
        b, half = divmod(core, 2)
        out[b][:, half * TQ:(half + 1) * TQ] = res.results[core]["outp"]
    return out.reshape(4, C, 64, 64)

